# revision 1
# baseline (speedup 1.0000x reference)
"""CodeGen-style attention block, tensor-parallel over heads on 8 Trainium2 cores.

Strategy (megatron-style):
  - Each core owns 2 of the 16 heads: computes Q/K/V projections for its
    head-slice of w_qkv, runs causal attention for those heads, then applies
    its row-slice of w_out, producing a partial [tokens, H] output.
  - Host sums the 8 partial outputs (the out-proj contraction over heads).

On-chip layout choices:
  - Everything is computed in the "transposed" domain: qT/kT [d, token] come
    straight out of the projection (weights stationary, hidden^T moving), so
    the score matmul scoresT[k, q] = kT_chunk.T @ qT needs no transposes.
  - Softmax runs along the partition (k) axis: exp on ScalarE (mask folded in
    via additive tiles + per-key bias), the denominator via a ones-column
    matmul, normalization deferred to after A@V (per-q reciprocal broadcast
    with a K=1 ones matmul).
  - V is produced transposed like q/k, then flipped to [k, d] with PE
    transposes (needed as the stationary side of A@V).
  - Matmuls use float32r (~1e-4 rel err, ~bf16 speed at N>=256).
  - Causal block-skipping: score/AV work for fully-masked k-chunks is skipped.
"""

import sys
import types
from contextlib import ExitStack

import numpy as np

import concourse.bacc as bacc
import concourse.mybir as mybir
import concourse.tile as tile
from concourse.bass_utils import run_bass_kernel_spmd

# bass_utils imports antenv.axon_hooks when tracing is requested via env;
# provide a no-op stub if the module is absent so a stray BASS_TRACE in the
# environment cannot break execution.
try:
    import antenv.axon_hooks  # noqa: F401
except ImportError:
    _stub = types.ModuleType("antenv.axon_hooks")
    _stub.get_axon_ntff_profile_hook = lambda: None
    _stub.set_axon_ntff_profile_hook = lambda h: None
    sys.modules.setdefault("antenv.axon_hooks", _stub)

F32 = mybir.dt.float32
F32R = mybir.dt.float32r
AF = mybir.ActivationFunctionType

B, S, H = 2, 2048, 4096
N_HEAD, HEAD_DIM, ROT = 16, 256, 64
MAX_POS = 2048
TOK = B * S            # 4096
N_CORES = 8
HPC = N_HEAD // N_CORES  # heads per core = 2
DPC = HPC * HEAD_DIM     # dims per core = 512
NEG = -30000.0

LAST_EXEC_NS = None
_NC_CACHE = []


def _build():
    nc = bacc.Bacc("TRN2", target_bir_lowering=False, debug=False,
                   num_devices=N_CORES)

    # [w, p, hc*256+t]: hsT tiles, per-partition-contiguous
    hst_d = nc.dram_tensor("hst", [16, 128, 32 * 256], F32R, kind="ExternalInput")
    # [oc, p, hc*128+d]: per-core w_qkv column-chunks (q0..3 k0..3 v0..3)
    wqkv_d = nc.dram_tensor("wqkv", [12, 128, 32 * 128], F32R, kind="ExternalInput")
    # [p, c, n]: per-core w_out row-slice
    wout_d = nc.dram_tensor("wout", [128, 4, H], F32R, kind="ExternalInput")
    rope_d = nc.dram_tensor("rope", [128, TOK], F32, kind="ExternalInput")
    rt_d = nc.dram_tensor("rt", [64, 64], F32R, kind="ExternalInput")
    id_d = nc.dram_tensor("ident", [128, 128], F32, kind="ExternalInput")
    onm_d = nc.dram_tensor("onesm", [128, 128], F32R, kind="ExternalInput")
    msk_d = nc.dram_tensor("masks", [128, 4, 512], F32, kind="ExternalInput")
    kb_d = nc.dram_tensor("kb", [128, 32], F32, kind="ExternalInput")
    out_d = nc.dram_tensor("out", [TOK, H], F32, kind="ExternalOutput")

    # phase-1 oc order: v and k first so attention inputs for batch 0 are
    # ready while the q projections still run; q last.
    PASS_OCS = ((8, 9, 10, 11, 4, 5), (6, 7, 0, 1, 2, 3))

    with tile.TileContext(nc) as tc:
        with ExitStack() as st0:
            ec0 = st0.enter_context
            dram_pool = ec0(tc.tile_pool(name="dram", bufs=1, space="DRAM"))
            # per-(oc, batch) intermediates so phase-2 loads only wait on the
            # phase-1 windows they actually read
            qkvT = {}
            for oc in range(12):
                for b in range(2):
                    qkvT[(oc, b)] = dram_pool.tile(
                        [128, 2048], F32R, tag=f"qkvT{oc}_{b}",
                        name=f"qkvT{oc}_{b}")
            # small phase-2 constants loaded up-front (DMA is idle-ish early)
            c2 = ec0(tc.tile_pool(name="p2c", bufs=1))
            msk_sb = c2.tile([128, 4, 512], F32)
            nc.sync.dma_start(msk_sb[:], msk_d[:])
            kb_sb = c2.tile([128, 32], F32)
            nc.sync.dma_start(kb_sb[:], kb_d[:])
            id_sb = c2.tile([128, 128], F32)
            nc.sync.dma_start(id_sb[:], id_d[:])
            onm_sb = c2.tile([128, 128], F32R)
            nc.sync.dma_start(onm_sb[:], onm_d[:])

            # ---------------- Phase 1: QKV projection + rotary ----------------
            with ExitStack() as st1:
                ec = st1.enter_context
                cpool = ec(tc.tile_pool(name="p1c", bufs=1))
                wpool = ec(tc.tile_pool(name="w", bufs=1))
                hpool = ec(tc.tile_pool(name="ht", bufs=2))
                spool = ec(tc.tile_pool(name="stage", bufs=6))
                tpool = ec(tc.tile_pool(name="rott", bufs=4))
                apool = ec(tc.tile_pool(name="acc", bufs=4, space="PSUM"))
                rpool = ec(tc.tile_pool(name="rp", bufs=2, space="PSUM"))
                rope_sb = cpool.tile([128, TOK], F32)
                rt_sb = cpool.tile([64, 64], F32R)

                def load_w(ocs, j0=0):
                    wts = []
                    for j, oc in enumerate(ocs):
                        wt = wpool.tile([128, 32 * 128], F32R, tag=f"w{j0 + j}",
                                        name=f"wt{j0 + j}")
                        nc.sync.dma_start(wt[:], wqkv_d[oc])
                        wts.append(wt)
                    return wts

                def ht_load(w, strips):
                    # strip the transfer so the first H-chunks land (and the
                    # first matmuls start) before the whole 8MB tile arrives
                    t = hpool.tile([128, 32 * 256], F32R, name="ht")
                    step = 32 // strips
                    for s in range(strips):
                        cs = slice(s * step * 256, (s + 1) * step * 256)
                        nc.sync.dma_start(t[:, cs], hst_d[w][:, cs])
                    return t

                wts = load_w(PASS_OCS[0][:1])  # w0 first: first MMs need it
                ht = ht_load(0, 4)
                wts += load_w(PASS_OCS[0][1:], j0=1)
                nc.sync.dma_start(rope_sb[:], rope_d[:])
                nc.sync.dma_start(rt_sb[:], rt_d[:])
                for p, ocs in enumerate(PASS_OCS):
                    if p > 0:
                        wts = next_wts
                        ht = next_ht
                    for w in range(16):
                        if w > 0:
                            ht = next_ht
                        ws = slice(w * 256, (w + 1) * 256)
                        wb, wo = w // 8, (w % 8) * 256
                        for j, oc in enumerate(ocs):
                            if j == 1:
                                # prefetch next window under this one's
                                # compute; strip the early windows so partial
                                # tiles unblock matmuls during the startup
                                # DMA backlog
                                if w < 15:
                                    next_ht = ht_load(w + 1, 2 if w < 4 else 1)
                                elif p + 1 < len(PASS_OCS):
                                    next_ht = ht_load(0, 2)
                            acc = apool.tile([128, 256], F32)
                            for hc in range(32):
                                nc.tensor.matmul(
                                    acc[:],
                                    wts[j][:, hc * 128:(hc + 1) * 128],
                                    ht[:, hc * 256:(hc + 1) * 256],
                                    start=(hc == 0), stop=(hc == 31),
                                )
                            stage = spool.tile([128, 256], F32R)
                            nc.vector.tensor_copy(stage[:], acc[:])
                            if oc in (0, 2, 4, 6):
                                # partial rotary on first 64 dims of this head
                                rp = rpool.tile([64, 256], F32)
                                nc.tensor.matmul(rp[:], rt_sb[:], stage[0:64, :])
                                t1 = tpool.tile([64, 256], F32, tag="t1")
                                nc.vector.tensor_mul(
                                    t1[:], acc[0:64, :], rope_sb[0:64, ws])
                                t2 = tpool.tile([64, 256], F32, tag="t2")
                                nc.vector.tensor_mul(
                                    t2[:], rp[:], rope_sb[64:128, ws])
                                nc.vector.tensor_add(stage[0:64, :], t1[:], t2[:])
                            nc.sync.dma_start(
                                qkvT[(oc, wb)][:, wo:wo + 256], stage[:])
                        if w == 15 and p + 1 < len(PASS_OCS):
                            # issue next pass's weight DMAs under this
                            # window's remaining compute
                            next_wts = load_w(PASS_OCS[p + 1])

            # ---------------- Phase 2: attention + out-proj ----------------
            with ExitStack() as st2:
                ec = st2.enter_context
                c3 = ec(tc.tile_pool(name="p2w", bufs=1))
                kpool = ec(tc.tile_pool(name="kt", bufs=1))
                vtpool = ec(tc.tile_pool(name="vt", bufs=2))
                vhpool = ec(tc.tile_pool(name="vh", bufs=1))
                qpool = ec(tc.tile_pool(name="qq", bufs=2))
                expool = ec(tc.tile_pool(name="ex", bufs=4))
                recpool = ec(tc.tile_pool(name="rec", bufs=2))
                aopool = ec(tc.tile_pool(name="ao", bufs=2))
                ospool = ec(tc.tile_pool(name="os", bufs=3))
                scpool = ec(tc.tile_pool(name="sc", bufs=2, space="PSUM"))
                avpool = ec(tc.tile_pool(name="av", bufs=1, space="PSUM"))
                denpool = ec(tc.tile_pool(name="den", bufs=2, space="PSUM"))
                oppool = ec(tc.tile_pool(name="op", bufs=2, space="PSUM"))
                wout_sb = c3.tile([128, 4, H], F32R)

                def emit_outproj(b, qt, aos):
                    qo = qt * 512
                    for tc_ in range(4):
                        for ht_ in range(8):
                            op = oppool.tile([128, 512], F32, tag="op")
                            for ci, (hl, dc) in enumerate(
                                    ((0, 0), (0, 1), (1, 0), (1, 1))):
                                nc.tensor.matmul(
                                    op[:],
                                    aos[(hl, dc)][:, tc_ * 128:(tc_ + 1) * 128],
                                    wout_sb[:, 2 * hl + dc,
                                            ht_ * 512:(ht_ + 1) * 512],
                                    start=(ci == 0), stop=(ci == 3))
                            os_ = ospool.tile([128, 512], F32)
                            nc.vector.tensor_copy(os_[:], op[:])
                            r0 = b * 2048 + qo + tc_ * 128
                            nc.sync.dma_start(
                                out_d[r0:r0 + 128, ht_ * 512:(ht_ + 1) * 512],
                                os_[:])

                pending = None
                for b in range(2):
                    kts = {}
                    vhs = {}
                    for hl in range(2):
                        vh = vhpool.tile([128, 16 * 256], F32R, tag=f"vh{hl}")
                        for dc in range(2):
                            vt = vtpool.tile([128, 2048], F32)
                            nc.sync.dma_start(
                                vt[:], qkvT[(8 + 2 * hl + dc, b)][:].bitcast(F32))
                            for kc in range(16):
                                tp = oppool.tile([128, 128], F32, tag="op")
                                nc.tensor.transpose(
                                    tp[:], vt[:, kc * 128:(kc + 1) * 128], id_sb[:])
                                nc.vector.tensor_copy(
                                    vh[:, kc * 256 + dc * 128:
                                       kc * 256 + (dc + 1) * 128], tp[:])
                        vhs[hl] = vh
                        for dc in range(2):
                            kt = kpool.tile([128, 2048], F32R, tag=f"kt{hl}{dc}")
                            nc.sync.dma_start(kt[:], qkvT[(4 + 2 * hl + dc, b)][:])
                            kts[(hl, dc)] = kt
                    for qt in range(4):
                        nkc = 4 * qt + 4  # causal: k-chunks beyond are all-masked
                        qo = qt * 512
                        aos = {}
                        for hl in range(2):
                            qs = []
                            for dc in range(2):
                                q = qpool.tile([128, 512], F32R, tag=f"q{dc}")
                                nc.sync.dma_start(
                                    q[:], qkvT[(2 * hl + dc, b)][:, qo:qo + 512])
                                qs.append(q)
                            if b == 0 and qt == 0 and hl == 0:
                                # out-proj weights are first needed one
                                # qt-block in; issue this 8MB DMA after the
                                # first attention inputs, not before
                                nc.sync.dma_start(wout_sb[:], wout_d[:])
                            av0 = avpool.tile([128, 512], F32, tag="av0")
                            av1 = avpool.tile([128, 512], F32, tag="av1")
                            den = denpool.tile([128, 512], F32)
                            for kc in range(nkc):
                                sc = scpool.tile([128, 512], F32)
                                nc.tensor.matmul(
                                    sc[:], kts[(hl, 0)][:, kc * 128:(kc + 1) * 128],
                                    qs[0][:], start=True, stop=False)
                                nc.tensor.matmul(
                                    sc[:], kts[(hl, 1)][:, kc * 128:(kc + 1) * 128],
                                    qs[1][:], start=False, stop=True)
                                if kc >= 4 * qt:
                                    nc.vector.tensor_add(
                                        sc[:], sc[:], msk_sb[:, kc - 4 * qt, :])
                                ex = expool.tile([128, 512], F32R)
                                nc.scalar.activation(
                                    ex[:], sc[:], AF.Exp, scale=1.0 / 16.0,
                                    bias=kb_sb[:, b * 16 + kc:b * 16 + kc + 1])
                                nc.tensor.matmul(
                                    av0[:], vhs[hl][:, kc * 256:kc * 256 + 128],
                                    ex[:], start=(kc == 0), stop=(kc == nkc - 1))
                                nc.tensor.matmul(
                                    av1[:], vhs[hl][:, kc * 256 + 128:kc * 256 + 256],
                                    ex[:], start=(kc == 0), stop=(kc == nkc - 1))
                                # denominator, pre-broadcast across partitions:
                                # ones[128,128].T @ ex = colsum replicated 128x
                                nc.tensor.matmul(
                                    den[:], onm_sb[:], ex[:],
                                    start=(kc == 0), stop=(kc == nkc - 1))
                            # fast av-bank evacuation on ScalarE (DVE's in-order queue is
                            # occupied by the ~3.4us reciprocal); den keeps its
                            # bank through the reciprocal (bufs=2 covers it)
                            avs = []
                            for dc, av in ((0, av0), (1, av1)):
                                avc = aopool.tile([128, 512], F32, bufs=1,
                                                  tag=f"avs{hl}{dc}", name="avc")
                                nc.scalar.copy(avc[:], av[:])
                                avs.append(avc)
                            rec = recpool.tile([128, 512], F32, tag="rec", bufs=1)
                            nc.vector.reciprocal(rec[:], den[:])
                            for dc in range(2):
                                ao = aopool.tile([128, 512], F32R, tag=f"ao{hl}{dc}")
                                nc.vector.tensor_mul(ao[:], avs[dc][:], rec[:])
                                aos[(hl, dc)] = ao
                        # software pipeline: emit the PREVIOUS block's out-proj
                        # here so its matmuls sit behind this block's attention
                        # in PE program order and never wait on normalization
                        if pending is not None:
                            emit_outproj(*pending)
                        pending = (b, qt, aos)
                emit_outproj(*pending)
    nc.compile()
    return nc


def _get_nc():
    if not _NC_CACHE:
        _NC_CACHE.append(_build())
    return _NC_CACHE[0]


def _host_prep(hidden_states, position_ids, attention_mask, w_qkv, w_out):
    hid = np.ascontiguousarray(np.asarray(hidden_states, np.float32)).reshape(TOK, H)
    w_qkv = np.asarray(w_qkv, np.float32)
    w_out = np.asarray(w_out, np.float32)
    pos = np.asarray(position_ids).astype(np.int64)
    am = np.asarray(attention_mask).reshape(B, S).astype(bool)

    # hsT tiles [w, p, hc*256+t]
    hst = np.ascontiguousarray(
        hid.reshape(16, 256, 32, 128).transpose(0, 3, 2, 1)).reshape(16, 128, 32 * 256)

    # rotary tables, matching reference.create_sinusoidal_positions
    inv_freq = 1.0 / 10000 ** (np.arange(0, ROT, 2) / ROT)
    si = np.einsum('i,j->ij', np.arange(MAX_POS), inv_freq).astype('float32')
    emb = np.concatenate([np.sin(si), np.cos(si)], axis=-1)  # [2048, 64]
    sincos = emb[pos]                    # [B, S, 64]
    sin_rep = np.repeat(sincos[..., :ROT // 2], 2, axis=2)   # [B, S, 64]
    cos_rep = np.repeat(sincos[..., ROT // 2:], 2, axis=2)
    rope = np.empty((128, TOK), np.float32)
    rope[0:64] = cos_rep.reshape(TOK, 64).T
    rope[64:128] = sin_rep.reshape(TOK, 64).T

    rt = np.zeros((64, 64), np.float32)
    rt[np.arange(1, 64, 2), np.arange(0, 64, 2)] = -1.0
    rt[np.arange(0, 64, 2), np.arange(1, 64, 2)] = 1.0

    ident = np.eye(128, dtype=np.float32)
    onesm = np.ones((128, 128), np.float32)

    p_idx = np.arange(128)[:, None, None]
    i_idx = np.arange(4)[None, :, None]
    q_idx = np.arange(512)[None, None, :]
    masks = np.where(p_idx + i_idx * 128 <= q_idx, 0.0, NEG).astype(np.float32)

    kb = np.where(am.reshape(B, 16, 128), 0.0, NEG).astype(
        np.float32).transpose(2, 0, 1).reshape(128, 32)
    kb = np.ascontiguousarray(kb)

    shared = dict(hst=hst, rope=rope, rt=rt, ident=ident, onesm=onesm,
                  masks=masks, kb=kb)

    in_maps = []
    for c in range(N_CORES):
        cols = []
        for part in (0, 2, 1):  # fused layout per mp-group is (query, value, key)
            for hl in range(HPC):
                h = HPC * c + hl
                base = (h // 4) * 3072 + part * 1024 + (h % 4) * 256
                cols.append(np.arange(base, base + 256))
        cols = np.concatenate(cols)  # [1536] = q(512) | k(512) | v(512)
        wslice = w_qkv[:, cols]      # [4096, 1536]
        wqkv_prep = np.ascontiguousarray(
            wslice.reshape(32, 128, 12, 128).transpose(2, 1, 0, 3)
        ).reshape(12, 128, 32 * 128)
        wout_prep = np.ascontiguousarray(
            w_out[c * DPC:(c + 1) * DPC, :].reshape(4, 128, H).transpose(1, 0, 2))
        in_maps.append(dict(shared, wqkv=wqkv_prep, wout=wout_prep))
    return in_maps


def kernel(hidden_states, position_ids, attention_mask, w_qkv, w_out):
    global LAST_EXEC_NS
    nc = _get_nc()
    in_maps = _host_prep(hidden_states, position_ids, attention_mask,
                         w_qkv, w_out)
    res = run_bass_kernel_spmd(nc, in_maps, core_ids=list(range(N_CORES)))
    LAST_EXEC_NS = res.exec_time_ns
    out = res.results[0]["out"].astype(np.float32)
    for c in range(1, N_CORES):
        out = out + res.results[c]["out"]
    return out.reshape(B, S, H)



# revision 2
# speedup vs baseline: 1.0123x; 1.0123x over previous
"""CodeGen-style attention block, tensor-parallel over heads on 8 Trainium2 cores.

v2: all-bf16 storage/matmuls (f32 PSUM accumulate), serial phase structure
P0 -> A0 -> P1 -> A1 -> OP:
  - P(b): single-pass QKV projection for batch b, 256-token windows, all 12
    weight column-tiles resident (bf16 halves SBUF; FWL hides LDWEIGHTS).
    k and v are written straight into persistent SBUF tiles (no DRAM
    round-trip); q goes to DRAM and is read back per query-block.  v is
    computed directly in [tok, d] form by swapping the matmul operands
    (hsT chunk stationary, v-weights moving) - no PE transposes.
  - A(b): causal attention for this core's 2 heads.  Scores/exp/AV/den with
    diagonal narrowing (only the live column range of diagonal k-chunks is
    computed) and depth-2 software pipelining of score vs AV matmuls in the
    PE stream.  Normalization is deferred: unnormalized AV sums (bf16) and
    softmax denominators (f32) are written to DRAM.
  - OP: out-projection.  Loads AV/den back, reciprocal (fast approx) +
    normalize on DVE off the critical path, then the w_out row-slice matmuls.
Host sums the 8 partial [tokens, H] outputs (out-proj head contraction).
"""

import sys
import types
from contextlib import ExitStack

import numpy as np
import ml_dtypes

import concourse.bacc as bacc
import concourse.mybir as mybir
import concourse.tile as tile
from concourse.bass_utils import run_bass_kernel_spmd

try:
    import antenv.axon_hooks  # noqa: F401
except ImportError:
    _stub = types.ModuleType("antenv.axon_hooks")
    _stub.get_axon_ntff_profile_hook = lambda: None
    _stub.set_axon_ntff_profile_hook = lambda h: None
    sys.modules.setdefault("antenv.axon_hooks", _stub)

F32 = mybir.dt.float32
BF = mybir.dt.bfloat16
AF = mybir.ActivationFunctionType
NPBF = ml_dtypes.bfloat16

B, S, H = 2, 2048, 4096
N_HEAD, HEAD_DIM, ROT = 16, 256, 64
MAX_POS = 2048
TOK = B * S            # 4096
N_CORES = 8
HPC = N_HEAD // N_CORES  # heads per core = 2
DPC = HPC * HEAD_DIM     # dims per core = 512
NEG = -30000.0
NW = 16                  # 256-token windows

LAST_EXEC_NS = None
_NC_CACHE = []


def _build():
    nc = bacc.Bacc("TRN2", target_bir_lowering=False, debug=False,
                   num_devices=N_CORES)

    # [w, p(H-chunk), hc*256 + t]: hidden^T window tiles
    hst_d = nc.dram_tensor("hst", [NW, 128, 32 * 256], BF, kind="ExternalInput")
    # [oc, p(H-chunk), hc*128 + d]: q ocs 0..3 then k ocs 4..7 (stationary)
    wqkv_d = nc.dram_tensor("wqkv", [8, 128, 32 * 128], BF, kind="ExternalInput")
    # [p(H-chunk), hc*512 + vd]: v weights as the moving operand
    wv_d = nc.dram_tensor("wv", [128, 32 * 512], BF, kind="ExternalInput")
    rope_d = nc.dram_tensor("rope", [128, TOK], F32, kind="ExternalInput")
    rt_d = nc.dram_tensor("rt", [64, 64], BF, kind="ExternalInput")
    onm_d = nc.dram_tensor("onesm", [128, 128], BF, kind="ExternalInput")
    tri_d = nc.dram_tensor("tri", [128, 128], F32, kind="ExternalInput")
    kb_d = nc.dram_tensor("kb", [128, 32], F32, kind="ExternalInput")
    # [p, ci(=2*hl+dc), h]: per-core w_out row-slice
    wout_d = nc.dram_tensor("wout", [128, 4, H], BF, kind="ExternalInput")
    out_d = nc.dram_tensor("out", [TOK, H], BF, kind="ExternalOutput")

    with tile.TileContext(nc) as tc:
        with ExitStack() as st0:
            ec0 = st0.enter_context
            dram_pool = ec0(tc.tile_pool(name="dram", bufs=1, space="DRAM"))
            q_d = {}
            for b in range(B):
                for oc in range(4):
                    q_d[(b, oc)] = dram_pool.tile(
                        [128, 2048], BF, tag=f"qd{b}_{oc}", name=f"qd{b}_{oc}")
            avc_d = {}
            den_d = {}
            for b in range(B):
                for qt in range(4):
                    for hl in range(2):
                        den_d[(b, qt, hl)] = dram_pool.tile(
                            [128, 512], F32, tag=f"dnd{b}{qt}{hl}",
                            name=f"dnd{b}{qt}{hl}")
                        for dc in range(2):
                            avc_d[(b, qt, hl, dc)] = dram_pool.tile(
                                [128, 512], BF, tag=f"avd{b}{qt}{hl}{dc}",
                                name=f"avd{b}{qt}{hl}{dc}")

            # constants (small)
            c0 = ec0(tc.tile_pool(name="consts", bufs=1))
            onm_sb = c0.tile([128, 128], BF)
            nc.sync.dma_start(onm_sb[:], onm_d[:])
            tri_sb = c0.tile([128, 128], F32)
            nc.sync.dma_start(tri_sb[:], tri_d[:])
            kb_sb = c0.tile([128, 32], F32)
            nc.sync.dma_start(kb_sb[:], kb_d[:])
            rt_sb = c0.tile([64, 64], BF)
            nc.sync.dma_start(rt_sb[:], rt_d[:])
            # preload the exp ACT table under phase-0 compute (one tiny exp)
            dummy_sb = c0.tile([1, 8], BF)
            nc.scalar.activation(dummy_sb[:], kb_sb[0:1, 0:8], AF.Exp)

            # persistent per-batch attention inputs (reused b0 -> b1)
            kvp = ec0(tc.tile_pool(name="kv", bufs=1))
            kts = {(hl, dc): kvp.tile([128, 2048], BF, tag=f"kt{hl}{dc}",
                                      name=f"kt{hl}{dc}")
                   for hl in range(2) for dc in range(2)}
            vh = kvp.tile([128, 16 * 512], BF, tag="vh", name="vh")

            # ---------------- attention machinery ----------------
            qpool = ec0(tc.tile_pool(name="qq", bufs=2))

            def q_dmas(b, qt, hl):
                ts = []
                for dc in range(2):
                    q = qpool.tile([128, 512], BF, tag=f"q{dc}")
                    nc.sync.dma_start(
                        q[:], q_d[(b, 2 * hl + dc)][:, qt * 512:(qt + 1) * 512])
                    ts.append(q)
                return ts

            def attn_phase(ast, b, preq=None, hook=None):
                ec = ast.enter_context
                scpool = ec(tc.tile_pool(name=f"sc{b}", bufs=4, space="PSUM"))
                avpool = ec(tc.tile_pool(name=f"av{b}", bufs=1, space="PSUM"))
                dnpool = ec(tc.tile_pool(name=f"dn{b}", bufs=2, space="PSUM"))
                expool = ec(tc.tile_pool(name=f"ex{b}", bufs=5))
                aspool = ec(tc.tile_pool(name=f"as{b}", bufs=2))

                blocks = [(qt, hl) for qt in range(4) for hl in range(2)]
                qtiles = {}

                def load_q(blk):
                    qtiles[blk] = q_dmas(b, blk[0], blk[1])

                if preq is not None:
                    qtiles[blocks[0]] = preq
                else:
                    load_q(blocks[0])
                pend = None
                for bi, blk in enumerate(blocks):
                    qt, hl = blk
                    if hook is not None:
                        hook(bi)
                    if bi + 1 < len(blocks):
                        load_q(blocks[bi + 1])
                    qs = qtiles.pop(blk)
                    nkc = 4 * qt + 4
                    av0 = avpool.tile([128, 512], F32, tag="av0")
                    av1 = avpool.tile([128, 512], F32, tag="av1")
                    den = dnpool.tile([128, 512], F32, tag="den")
                    exs = {}

                    def avden(kc):
                        ex, n0 = exs.pop(kc)
                        st, sp = (kc == 0), (kc == nkc - 1)
                        base = kc * 512 + hl * 256
                        nc.tensor.matmul(
                            av0[:, n0:512], vh[:, base:base + 128],
                            ex[:, n0:512], start=st, stop=sp)
                        nc.tensor.matmul(
                            av1[:, n0:512], vh[:, base + 128:base + 256],
                            ex[:, n0:512], start=st, stop=sp)
                        nc.tensor.matmul(
                            den[:, n0:512], onm_sb[:],
                            ex[:, n0:512], start=st, stop=sp)

                    for kc in range(nkc):
                        j = kc - 4 * qt
                        n0 = 128 * j if j > 0 else 0
                        sc = scpool.tile([128, 512], F32, tag="sc")
                        nc.tensor.matmul(
                            sc[:, n0:512], kts[(hl, 0)][:, kc * 128:(kc + 1) * 128],
                            qs[0][:, n0:512], start=True, stop=False)
                        nc.tensor.matmul(
                            sc[:, n0:512], kts[(hl, 1)][:, kc * 128:(kc + 1) * 128],
                            qs[1][:, n0:512], start=False, stop=True)
                        if j >= 0:
                            # causal triangle within the diagonal 128-col band
                            nc.vector.tensor_add(
                                sc[:, n0:n0 + 128], sc[:, n0:n0 + 128], tri_sb[:])
                        ex = expool.tile([128, 512], BF, tag="ex")
                        nc.scalar.activation(
                            ex[:, n0:512], sc[:, n0:512], AF.Exp,
                            scale=1.0 / 16.0,
                            bias=kb_sb[:, b * 16 + kc:b * 16 + kc + 1])
                        exs[kc] = (ex, n0)
                        if kc == 1 and pend is not None:
                            pend()
                            pend = None
                        if kc >= 3:
                            avden(kc - 3)
                    avden(nkc - 3)
                    avden(nkc - 2)
                    avden(nkc - 1)

                    def make_fin(qt=qt, hl=hl, av0=av0, av1=av1, den=den):
                        def fin():
                            for dc, av in ((0, av0), (1, av1)):
                                avc = aspool.tile([128, 512], BF,
                                                  tag=f"avc{dc}")
                                nc.vector.tensor_copy(avc[:], av[:])
                                nc.sync.dma_start(
                                    avc_d[(b, qt, hl, dc)][:], avc[:])
                            dns = aspool.tile([128, 512], F32, tag="dns")
                            nc.vector.tensor_copy(dns[:], den[:])
                            nc.sync.dma_start(den_d[(b, qt, hl)][:], dns[:])
                        return fin

                    pend = make_fin()
                pend()

            # ---------------- phases P0, A0, P1 ----------------
            with ExitStack() as stw:
                ecw = stw.enter_context
                wpool = ecw(tc.tile_pool(name="w", bufs=1))
                htpool = ecw(tc.tile_pool(name="ht", bufs=2))
                ropool = ecw(tc.tile_pool(name="ro", bufs=2))
                spool = ecw(tc.tile_pool(name="stage", bufs=4))
                tpool = ecw(tc.tile_pool(name="rott", bufs=2))

                def ht_load(w, strips=1):
                    t = htpool.tile([128, 32 * 256], BF, tag="ht", name="ht")
                    step = 32 // strips
                    for s in range(strips):
                        cs = slice(s * step * 256, (s + 1) * step * 256)
                        nc.sync.dma_start(t[:, cs], hst_d[w][:, cs])
                    return t

                def rope_load(w):
                    t = ropool.tile([128, 256], F32, tag="ro", name="ro")
                    nc.sync.dma_start(t[:], rope_d[:, w * 256:(w + 1) * 256])
                    return t

                # weight loads: k first (first used), then v, then q
                wts = {}
                for kq in range(4):
                    wt = wpool.tile([128, 32 * 128], BF, tag=f"w{4 + kq}",
                                    name=f"wt{4 + kq}")
                    nc.sync.dma_start(wt[:], wqkv_d[4 + kq])
                    wts[4 + kq] = wt
                pre = (ht_load(0, 4), rope_load(0))
                wv_sb = wpool.tile([128, 32 * 512], BF, tag="wv", name="wv")
                nc.sync.dma_start(wv_sb[:], wv_d[:])
                for oc in range(4):
                    wt = wpool.tile([128, 32 * 128], BF, tag=f"w{oc}",
                                    name=f"wt{oc}")
                    nc.sync.dma_start(wt[:], wqkv_d[oc])
                    wts[oc] = wt

                def proj_phase(pst, b, pre):
                    """QKV projection for batch b (windows 8b .. 8b+7)."""
                    ec = pst.enter_context
                    apool = ec(tc.tile_pool(name=f"pa{b}", bufs=3, space="PSUM"))
                    rpool = ec(tc.tile_pool(name=f"pr{b}", bufs=2, space="PSUM"))
                    ht, ro = pre
                    nxt = None
                    preq = None
                    for wl in range(8):
                        w = 8 * b + wl
                        if wl > 0:
                            ht, ro = next_pre
                        if wl == 2:
                            # early q loads for this batch's first attn block
                            preq = q_dmas(b, 0, 0)

                        def rotary(dst, acc, stage_rows, ro=ro):
                            # dst[0:64] <- acc*cos + rotate_every_two(acc)*sin
                            rp = rpool.tile([64, 256], F32)
                            nc.tensor.matmul(rp[:], rt_sb[:], stage_rows,
                                             start=True, stop=True)
                            t1 = tpool.tile([64, 256], F32, tag="t1")
                            nc.vector.tensor_mul(t1[:], acc[0:64, :], ro[0:64, :])
                            t2 = tpool.tile([64, 256], F32, tag="t2")
                            nc.vector.tensor_mul(t2[:], rp[:], ro[64:128, :])
                            nc.vector.tensor_add(dst, t1[:], t2[:])

                        # k ocs (stationary weights, transposed domain)
                        for kq in range(4):
                            hl, dc = kq // 2, kq % 2
                            acc = apool.tile([128, 256], F32, tag="acc")
                            for hc in range(32):
                                nc.tensor.matmul(
                                    acc[:],
                                    wts[4 + kq][:, hc * 128:(hc + 1) * 128],
                                    ht[:, hc * 256:(hc + 1) * 256],
                                    start=(hc == 0), stop=(hc == 31))
                            dst = kts[(hl, dc)][:, wl * 256:(wl + 1) * 256]
                            nc.vector.tensor_copy(dst, acc[:])
                            if dc == 0:
                                rotary(dst[0:64, :], acc, dst[0:64, :])
                            if kq == 1:
                                # prefetch next window under this one
                                if wl < 7:
                                    next_pre = (ht_load(w + 1), rope_load(w + 1))
                                elif b == 0:
                                    nxt = (ht_load(8), rope_load(8))
                        # v (hsT chunks stationary, v-weights moving -> [tok, d])
                        for ts in range(2):
                            acc = apool.tile([128, 512], F32, tag="acc")
                            for hc in range(32):
                                nc.tensor.matmul(
                                    acc[:],
                                    ht[:, hc * 256 + ts * 128:
                                       hc * 256 + (ts + 1) * 128],
                                    wv_sb[:, hc * 512:(hc + 1) * 512],
                                    start=(hc == 0), stop=(hc == 31))
                            kc = wl * 2 + ts
                            nc.vector.tensor_copy(
                                vh[:, kc * 512:(kc + 1) * 512], acc[:])
                        # q ocs -> DRAM
                        for oc in range(4):
                            hl, dc = oc // 2, oc % 2
                            acc = apool.tile([128, 256], F32, tag="acc")
                            for hc in range(32):
                                nc.tensor.matmul(
                                    acc[:], wts[oc][:, hc * 128:(hc + 1) * 128],
                                    ht[:, hc * 256:(hc + 1) * 256],
                                    start=(hc == 0), stop=(hc == 31))
                            stage = spool.tile([128, 256], BF, tag="qs")
                            nc.vector.tensor_copy(stage[:], acc[:])
                            if dc == 0:
                                rotary(stage[0:64, :], acc, stage[0:64, :])
                            nc.sync.dma_start(
                                q_d[(b, oc)][:, wl * 256:(wl + 1) * 256],
                                stage[:])
                    return nxt, preq

                with ExitStack() as pst:
                    pre1, preq0 = proj_phase(pst, 0, pre)
                with ExitStack() as ast:
                    attn_phase(ast, 0, preq=preq0)
                with ExitStack() as pst:
                    _, preq1 = proj_phase(pst, 1, pre1)
            # w_scope closed: projection SBUF freed

            with ExitStack() as st3:
                ec3 = st3.enter_context
                c3 = ec3(tc.tile_pool(name="wo", bufs=1))
                wout_sb = c3.tile([128, 4, H], BF)

                # ---------------- out-projection ----------------
                alpool = ec3(tc.tile_pool(name="al", bufs=2))
                dlpool = ec3(tc.tile_pool(name="dl", bufs=2))
                recpool = ec3(tc.tile_pool(name="rec", bufs=2))
                scrpool = ec3(tc.tile_pool(name="scr", bufs=2))
                aopool = ec3(tc.tile_pool(name="ao", bufs=2))
                ospool = ec3(tc.tile_pool(name="os", bufs=4))

                aos_all = {}

                def prep(b, qt):
                    aos = {}
                    for hl in range(2):
                        dn = dlpool.tile([128, 512], F32, tag=f"dn{hl}")
                        nc.sync.dma_start(dn[:], den_d[(b, qt, hl)][:])
                        rc = recpool.tile([128, 512], F32, tag=f"rc{hl}")
                        scr = scrpool.tile([128, 512], F32, tag="scr")
                        nc.vector.reciprocal_approx_accurate(
                            out=rc[:], in_=dn[:], scratch=scr[:])
                        for dc in range(2):
                            al = alpool.tile([128, 512], BF, tag=f"al{hl}{dc}")
                            nc.sync.dma_start(al[:], avc_d[(b, qt, hl, dc)][:])
                            ao = aopool.tile([128, 512], BF, tag=f"ao{hl}{dc}")
                            # gpsimd: keep DVE free for the attention stream
                            nc.gpsimd.tensor_mul(ao[:], al[:], rc[:])
                            aos[(hl, dc)] = ao
                    aos_all[(b, qt)] = aos

                def op_block(b, qt):
                    aos = aos_all.pop((b, qt))
                    for tc_ in range(4):
                        for ht_ in range(8):
                            op = oppool.tile([128, 512], F32, tag="op")
                            for ci, (hl, dc) in enumerate(
                                    ((0, 0), (0, 1), (1, 0), (1, 1))):
                                nc.tensor.matmul(
                                    op[:],
                                    aos[(hl, dc)][:, tc_ * 128:(tc_ + 1) * 128],
                                    wout_sb[:, 2 * hl + dc,
                                            ht_ * 512:(ht_ + 1) * 512],
                                    start=(ci == 0), stop=(ci == 3))
                            os_ = ospool.tile([128, 512], BF, tag="os")
                            if ht_ % 2 == 0:
                                nc.vector.tensor_copy(os_[:], op[:])
                            else:
                                nc.scalar.copy(os_[:], op[:])
                            r0 = b * 2048 + qt * 512 + tc_ * 128
                            nc.sync.dma_start(
                                out_d[r0:r0 + 128, ht_ * 512:(ht_ + 1) * 512],
                                os_[:])

                def a1_hook(bi):
                    # stagger the wout load and the first two out-proj preps
                    # under A1 so the OP phase starts with zero stalls
                    if bi == 1:
                        nc.sync.dma_start(wout_sb[:, 0:2, :], wout_d[:, 0:2, :])
                    elif bi == 3:
                        nc.sync.dma_start(wout_sb[:, 2:4, :], wout_d[:, 2:4, :])
                    elif bi == 2:
                        prep(0, 0)
                    elif bi == 5:
                        prep(0, 1)

                with ExitStack() as ast:
                    attn_phase(ast, 1, preq=preq1, hook=a1_hook)

                oppool = ec3(tc.tile_pool(name="op", bufs=4, space="PSUM"))
                obs = [(b, qt) for b in range(B) for qt in range(4)]
                for i, ob in enumerate(obs):
                    op_block(*ob)
                    if i + 2 < len(obs):
                        prep(*obs[i + 2])
    nc.compile()
    return nc


def _get_nc():
    if not _NC_CACHE:
        _NC_CACHE.append(_build())
    return _NC_CACHE[0]


def _host_prep(hidden_states, position_ids, attention_mask, w_qkv, w_out):
    hid = np.ascontiguousarray(np.asarray(hidden_states, np.float32)).reshape(TOK, H)
    w_qkv = np.asarray(w_qkv, np.float32)
    w_out = np.asarray(w_out, np.float32)
    pos = np.asarray(position_ids).astype(np.int64)
    am = np.asarray(attention_mask).reshape(B, S).astype(bool)

    # hsT window tiles [w, p, hc*256 + t]
    hst = np.ascontiguousarray(
        hid.astype(NPBF).reshape(NW, 256, 32, 128).transpose(0, 3, 2, 1)
    ).reshape(NW, 128, 32 * 256)

    # rotary tables, matching reference.create_sinusoidal_positions
    inv_freq = 1.0 / 10000 ** (np.arange(0, ROT, 2) / ROT)
    si = np.einsum('i,j->ij', np.arange(MAX_POS), inv_freq).astype('float32')
    emb = np.concatenate([np.sin(si), np.cos(si)], axis=-1)  # [2048, 64]
    sincos = emb[pos]                    # [B, S, 64]
    sin_rep = np.repeat(sincos[..., :ROT // 2], 2, axis=2)   # [B, S, 64]
    cos_rep = np.repeat(sincos[..., ROT // 2:], 2, axis=2)
    rope = np.empty((128, TOK), np.float32)
    rope[0:64] = cos_rep.reshape(TOK, 64).T
    rope[64:128] = sin_rep.reshape(TOK, 64).T

    rt = np.zeros((64, 64), np.float32)
    rt[np.arange(1, 64, 2), np.arange(0, 64, 2)] = -1.0
    rt[np.arange(0, 64, 2), np.arange(1, 64, 2)] = 1.0

    onesm = np.ones((128, 128), np.float32)

    p_idx = np.arange(128)[:, None]
    c_idx = np.arange(128)[None, :]
    tri = np.where(p_idx <= c_idx, 0.0, NEG).astype(np.float32)

    kb = np.where(am.reshape(B, 16, 128), 0.0, NEG).astype(
        np.float32).transpose(2, 0, 1).reshape(128, 32)
    kb = np.ascontiguousarray(kb)

    shared = dict(hst=hst, rope=rope, rt=rt.astype(NPBF),
                  onesm=onesm.astype(NPBF), tri=tri, kb=kb)

    in_maps = []
    for c in range(N_CORES):
        # q ocs 0..3 then k ocs 4..7; fused layout per mp-group is (q, v, k)
        occols = []
        for part in (0, 2):  # 0 = query, 2 = key
            for hl in range(HPC):
                h = HPC * c + hl
                base = (h // 4) * 3072 + part * 1024 + (h % 4) * 256
                occols.append(np.arange(base, base + 256))
        occols = np.concatenate(occols)  # [1024] = q(512) | k(512)
        wslice = w_qkv[:, occols].astype(NPBF)  # [4096, 1024]
        wqkv_prep = np.ascontiguousarray(
            wslice.reshape(32, 128, 8, 128).transpose(2, 1, 0, 3)
        ).reshape(8, 128, 32 * 128)

        vcols = []
        for hl in range(HPC):
            h = HPC * c + hl
            base = (h // 4) * 3072 + 1 * 1024 + (h % 4) * 256
            vcols.append(np.arange(base, base + 256))
        vcols = np.concatenate(vcols)    # [512]
        wv_prep = np.ascontiguousarray(
            w_qkv[:, vcols].astype(NPBF).reshape(32, 128, 512).transpose(1, 0, 2)
        ).reshape(128, 32 * 512)

        wout_prep = np.ascontiguousarray(
            w_out[c * DPC:(c + 1) * DPC, :].astype(NPBF)
            .reshape(4, 128, H).transpose(1, 0, 2))
        in_maps.append(dict(shared, wqkv=wqkv_prep, wv=wv_prep,
                            wout=wout_prep))
    return in_maps


def kernel(hidden_states, position_ids, attention_mask, w_qkv, w_out):
    global LAST_EXEC_NS
    nc = _get_nc()
    in_maps = _host_prep(hidden_states, position_ids, attention_mask,
                         w_qkv, w_out)
    res = run_bass_kernel_spmd(nc, in_maps, core_ids=list(range(N_CORES)))
    LAST_EXEC_NS = res.exec_time_ns
    out = res.results[0]["out"].astype(np.float32)
    for c in range(1, N_CORES):
        out = out + res.results[c]["out"].astype(np.float32)
    return out.reshape(B, S, H)
